# revision 1
# baseline (speedup 1.0000x reference)
"""BatchedKiloNeRF Trainium2 kernel.

Strategy (expert-parallel, host routing):
  - 4096 tiny MLPs ("experts"), 131072 points routed by model_indices.
  - Host sorts experts by point count, packs them into groups of 16
    (8 cores x 32 groups x 16 experts). Each group of 16 experts maps onto
    the 16 independent 32x32 sub-arrays of the PE (tile_position packing):
    expert (row r, col c) reads rhs from partition band 32r and writes
    PSUM band 32c, so all 16 per-expert matmuls run concurrently.
  - HW constraint (found empirically): concurrent matmuls from different
    row groups must not write the same PSUM bank. Each step therefore uses
    a 4-bank PSUM tile [128, 2048]; row group r writes its outputs into
    bank r (column window 512*r..512*r+C). Copies back to SBUF use strided
    APs so each step needs only one ACT/DVE op.
  - Points of each expert are padded to the group capacity C (max count in
    the 128-expert window); hidden states live as [128, 4C] SBUF tiles:
    partition band = expert band, C-column segment = expert segment.
  - Layer chain alternates layouts: A (band=e%4, seg=e//4) <-> B
    (band=e//4, seg=e%4); every step uses all 16 (row, col) positions.
  - Biases: L0 and view-layer biases ride in the matmul via a constant-1
    input row (K=3 -> K=4). feat bias is folded into the view bias on the
    host. L1 bias+relu is a fused DVE tensor_scalar (add, max) per bank
    slot. sigma/rgb biases are added on host.
  - Weights are pre-permuted/transposed on host into per-group SBUF-layout
    blobs so each group needs only a handful of large DMAs.
"""

import sys

import numpy as np

for _p in ("/opt/trn_rl_repo",):
    if _p not in sys.path:
        sys.path.append(_p)

NUM_MODELS = 4096
W = 32
N = 131072
NCORES = 8
NGROUPS = 32          # expert groups per core
EPG = 16              # experts per group
WIN = NCORES * EPG    # experts per capacity window (128)

# wblob column layout (per group, [128, WBLOB_F]):
#   w1 lhsT    [  0:128)
#   viewA lhsT [128:256)   (view_w[:, :32] @ feat_w folded on host)
#   sigma lhsT [256:260)
#   rgb lhsT   [260:272)
#   b1 bias    [272:276)   (per-slot per-partition bias columns)
#   zero pad   [276:304)   (SIM_SAFE mode widens sigma/rgb lhsT windows to
#                           M=32 so the full PSUM band is written)
WBLOB_F = 304
# SIM_SAFE: write full 32-row PSUM bands for sigma/rgb so CoreSim's
# uninitialized-read checker passes; on HW the junk rows are never read and
# narrow loads (M=1 / M=3) are ~25us faster.
SIM_SAFE = False
# sblob per group [16, 256]: w0aug lhsT [0:128), viewBaug lhsT [128:256)
SBLOB_F = 256
BANK = 512            # f32 elements per PSUM bank (per partition)


def _prep(x, model_indices, pts_w0, pts_b0, pts_w1, pts_b1,
          feat_w, feat_b, sigma_w, sigma_b, view_w, view_b, rgb_w, rgb_b):
    """Host-side routing + packing. Returns per-core device arrays and
    decode info."""
    x = np.asarray(x, np.float32)
    idx = np.asarray(model_indices).astype(np.int64)
    counts = np.bincount(idx, minlength=NUM_MODELS)

    expert_order = np.argsort(-counts, kind="stable")  # descending count
    caps = np.empty(NGROUPS, np.int64)
    for k in range(NGROUPS):
        win = expert_order[WIN * k:WIN * (k + 1)]
        c = int(counts[win].max())
        caps[k] = max(4, -(-c // 4) * 4)  # round up to multiple of 4, >=4
    assert caps.max() <= BANK
    colstart = np.concatenate([[0], np.cumsum(4 * caps)])
    w_tot = int(colstart[-1])

    order_pts = np.argsort(idx, kind="stable")
    starts = np.concatenate([[0], np.cumsum(counts)])

    # fold the feat layer into the view layer on the host:
    #   view(h) = relu(Wv [feat(h); views] + bv)
    #           = relu((Wv[:, :32] @ Wf) h + WvB views + (bv + Wv[:, :32] bf))
    vb_fold = view_b + np.einsum("goh,gh->go", view_w[:, :, :W], feat_b)
    vwA_fold = np.einsum("gox,gxh->goh", view_w[:, :, :W], feat_w)
    w0aug = np.concatenate(
        [np.transpose(pts_w0, (0, 2, 1)), pts_b0[:, None, :]], axis=1
    ).astype(np.float32)                      # [E, 4, 32] lhsT rows: xyz+bias
    vwBaug = np.concatenate(
        [np.transpose(view_w[:, :, W:], (0, 2, 1)), vb_fold[:, None, :]], axis=1
    ).astype(np.float32)                      # [E, 4, 32]
    w1T = np.transpose(pts_w1, (0, 2, 1)).astype(np.float32)    # [E,32,32]
    vwAT = np.transpose(vwA_fold, (0, 2, 1)).astype(np.float32)
    sigT = np.transpose(sigma_w, (0, 2, 1)).astype(np.float32)  # [E,32,1]
    rgbT = np.transpose(rgb_w, (0, 2, 1)).astype(np.float32)    # [E,32,3]
    b1 = np.asarray(pts_b1, np.float32)

    per_core = []
    decode = []  # (core, gid, pts, row_b, col_lo, cnt)
    for c in range(NCORES):
        gq = np.stack([expert_order[WIN * k + EPG * c: WIN * k + EPG * (c + 1)]
                       for k in range(NGROUPS)])  # [32, 16]

        wblob = np.zeros((NGROUPS, 128, WBLOB_F), np.float32)
        # B-step matrices (input layout B): l = 4b+s -> [k,(b kin),(s kout)]
        arr = w1T[gq].reshape(NGROUPS, 4, 4, W, W)    # [k,b,s,kin,kout]
        wblob[:, :, 0:128] = arr.transpose(0, 1, 3, 2, 4).reshape(NGROUPS, 128, 128)
        arr = vwAT[gq].reshape(NGROUPS, 4, 4, W, W)  # [k,j,i,kin,kout] (A)
        wblob[:, :, 128:256] = arr.transpose(0, 2, 3, 1, 4).reshape(NGROUPS, 128, 128)
        # A-step matrices (input layout A): l = 4j+i -> [k,(i kin),(j kout)]
        arr = sigT[gq].reshape(NGROUPS, 4, 4, W, 1)
        wblob[:, :, 256:260] = arr.transpose(0, 2, 3, 1, 4).reshape(NGROUPS, 128, 4)
        arr = rgbT[gq].reshape(NGROUPS, 4, 4, W, 3)   # [k,b,s,kin,kout] (B)
        wblob[:, :, 260:272] = arr.transpose(0, 1, 3, 2, 4).reshape(NGROUPS, 128, 12)
        # b1: L1 out layout A: partition 32s+h (s=e%4), slot q=e//4 -> col q
        arr = b1[gq].reshape(NGROUPS, 4, 4, W)        # [k,q,s,h]
        wblob[:, :, 272:276] = arr.transpose(0, 2, 3, 1).reshape(NGROUPS, 128, 4)

        sblob = np.zeros((NGROUPS, 16, SBLOB_F), np.float32)
        arr = w0aug[gq].reshape(NGROUPS, 4, 4, 4, W)  # [k,j,i,kin,kout] (A)
        sblob[:, :, 0:128] = arr.transpose(0, 2, 3, 1, 4).reshape(NGROUPS, 16, 128)
        arr = vwBaug[gq].reshape(NGROUPS, 4, 4, 4, W)  # [k,j,i,kin,kout] (A)
        sblob[:, :, 128:256] = arr.transpose(0, 2, 3, 1, 4).reshape(NGROUPS, 16, 128)

        xpts = np.zeros((16, w_tot), np.float32)
        views = np.zeros((16, w_tot), np.float32)
        xpts[3::4, :] = 1.0   # constant-1 rows for bias-in-matmul
        views[3::4, :] = 1.0
        for k in range(NGROUPS):
            C = int(caps[k])
            for l in range(EPG):
                gid = int(gq[k, l])
                cnt = int(counts[gid])
                pts = order_pts[starts[gid]:starts[gid] + cnt]
                i, j = l % 4, l // 4      # layout A (band, seg)
                ca = int(colstart[k]) + C * j   # A-seg columns
                cs = int(colstart[k]) + C * i   # B-seg columns
                if cnt:
                    xv = x[pts]
                    xpts[4 * i:4 * i + 3, ca:ca + cnt] = xv[:, :3].T
                    views[4 * i:4 * i + 3, ca:ca + cnt] = xv[:, 3:6].T
                # rgb lands at rows 4i+0..2, cols ca; sigma at row 4j+3, cols cs
                decode.append((c, gid, pts, i, j, ca, cs, cnt))
        per_core.append(dict(
            xpts=xpts, views=views,
            wblob=wblob.transpose(1, 0, 2).reshape(128, NGROUPS * WBLOB_F),
            sblob=sblob.transpose(1, 0, 2).reshape(16, NGROUPS * SBLOB_F)))

    return per_core, decode, caps, colstart, w_tot


def _build_nc(caps, w_tot):
    import concourse.mybir as mybir
    import concourse.tile as tile
    from concourse import bacc
    from contextlib import ExitStack

    f32 = mybir.dt.float32
    RELU = mybir.ActivationFunctionType.Relu
    ADD = mybir.AluOpType.add
    MAX = mybir.AluOpType.max

    nc = bacc.Bacc("TRN2", target_bir_lowering=False)
    xpts_d = nc.declare_dram_parameter("xpts", [16, w_tot], f32, isOutput=False)
    views_d = nc.declare_dram_parameter("views", [16, w_tot], f32, isOutput=False)
    wblob_d = nc.declare_dram_parameter("wblob", [128, NGROUPS * WBLOB_F], f32,
                                        isOutput=False)
    sblob_d = nc.declare_dram_parameter("sblob", [16, NGROUPS * SBLOB_F], f32,
                                        isOutput=False)
    out_d = nc.declare_dram_parameter("out", [16, w_tot], f32, isOutput=True)

    with tile.TileContext(nc) as tc, ExitStack() as ctx:
        const = ctx.enter_context(tc.tile_pool(name="const", bufs=1))
        hpool = ctx.enter_context(tc.tile_pool(name="h", bufs=8))
        pspool = ctx.enter_context(tc.tile_pool(name="ps", bufs=1, space="PSUM"))
        # One persistent 8-bank PSUM tensor, hand-slotted: a step claims
        # (bank-set, column-offset) slot; bank q within the set = row group q
        # (different row groups must not share a bank; same row group may).
        psall = pspool.tile([128, 8 * BANK], f32, tag="psall")
        step_ctr = [0]

        def ps_step():
            sidx = step_ctr[0]
            step_ctr[0] += 1
            bs = sidx % 2
            co = ((sidx // 2) % 8) * 64

            def mm_out(part_lo, m, q, C):
                base = (4 * bs + q) * BANK + co
                return psall[part_lo:part_lo + m, base:base + C]

            def copy_src(C):
                return psall.rearrange("p (b w) -> p b w", b=8)[
                    :, 4 * bs:4 * bs + 4, co:co + C]

            return mm_out, copy_src

        xt = const.tile([128, w_tot], f32)
        vt = const.tile([128, w_tot], f32)
        for i in range(4):
            nc.sync.dma_start(out=xt[32 * i:32 * i + 4, :],
                              in_=xpts_d[4 * i:4 * i + 4, :])
            nc.sync.dma_start(out=vt[32 * i:32 * i + 4, :],
                              in_=views_d[4 * i:4 * i + 4, :])
        wt_all = const.tile([128, NGROUPS * WBLOB_F], f32)
        wtot = NGROUPS * WBLOB_F
        nchunk = 8
        csz = -(-wtot // nchunk)
        for u in range(nchunk):
            lo, hi = u * csz, min((u + 1) * csz, wtot)
            nc.sync.dma_start(out=wt_all[:, lo:hi], in_=wblob_d[:, lo:hi])
        st_all = const.tile([128, NGROUPS * SBLOB_F], f32)
        for i in range(4):
            nc.sync.dma_start(out=st_all[32 * i:32 * i + 4, :],
                              in_=sblob_d[4 * i:4 * i + 4, :])
        otr_all = const.tile([128, w_tot], f32)
        ots_all = const.tile([128, w_tot], f32)

        # Software-pipeline: emit steps step-major over windows of PIPE
        # groups so the PE always has another group's matmuls to run while
        # a step's PSUM->SBUF copy completes.
        PIPE = 4
        colstarts = np.concatenate([[0], np.cumsum(4 * np.asarray(caps))])

        def group_steps(g):
            C = int(caps[g])
            W4 = 4 * C
            col = int(colstarts[g])
            wt = wt_all[:, g * WBLOB_F:(g + 1) * WBLOB_F]
            st = st_all[:, g * SBLOB_F:(g + 1) * SBLOB_F]
            state = {}

            def s_l0():
                mm0, cp0 = ps_step()
                for j in range(4):
                    for i in range(4):
                        nc.tensor.matmul(
                            out=mm0(32 * j, 32, i, C),
                            lhsT=st[32 * i:32 * i + 4, 32 * j:32 * j + 32],
                            rhs=xt[32 * i:32 * i + 4, col + C * j:col + C * j + C],
                            start=True, stop=True, skip_group_check=True,
                            tile_position=(32 * i, 32 * j))
                h1 = hpool.tile([128, W4], f32, tag="h1")
                nc.scalar.activation(h1.rearrange("p (q w) -> p q w", q=4),
                                     cp0(C), RELU)
                state["h1"] = h1

            def s_l1():
                h1 = state.pop("h1")
                mm1, _ = ps_step()
                for s in range(4):
                    for b in range(4):
                        nc.tensor.matmul(
                            out=mm1(32 * s, 32, b, C),
                            lhsT=wt[32 * b:32 * b + 32, 32 * s:32 * s + 32],
                            rhs=h1[32 * b:32 * b + 32, C * s:C * s + C],
                            start=True, stop=True, skip_group_check=True,
                            tile_position=(32 * b, 32 * s))
                h2 = hpool.tile([128, W4], f32, tag="h2")
                for q in range(4):
                    nc.vector.tensor_scalar(
                        out=h2[:, C * q:C * q + C],
                        in0=mm1(0, 128, q, C),
                        scalar1=wt[:, 272 + q:273 + q], scalar2=0.0,
                        op0=ADD, op1=MAX)
                state["h2"] = h2

            def s_sigma():
                h2 = state["h2"]
                MS = 32 if SIM_SAFE else 1
                mms_, cps = ps_step()
                for j in range(4):
                    for i in range(4):
                        nc.tensor.matmul(
                            out=mms_(32 * j, MS, i, C),
                            lhsT=wt[32 * i:32 * i + 32, 256 + j:256 + j + MS],
                            rhs=h2[32 * i:32 * i + 32, C * j:C * j + C],
                            start=True, stop=True, skip_group_check=True,
                            tile_position=(32 * i, 32 * j))
                nc.scalar.copy(
                    ots_all[:, col:col + W4].rearrange("p (q w) -> p q w", q=4),
                    cps(C))

            def s_view():
                h2 = state.pop("h2")
                mmv, cpv = ps_step()
                for j in range(4):
                    for i in range(4):
                        nc.tensor.matmul(
                            out=mmv(32 * j, 32, i, C),
                            lhsT=wt[32 * i:32 * i + 32, 128 + 32 * j:128 + 32 * j + 32],
                            rhs=h2[32 * i:32 * i + 32, C * j:C * j + C],
                            start=True, stop=False, skip_group_check=True,
                            tile_position=(32 * i, 32 * j))
                    for i in range(4):
                        nc.tensor.matmul(
                            out=mmv(32 * j, 32, i, C),
                            lhsT=st[32 * i:32 * i + 4, 128 + 32 * j:128 + 32 * j + 32],
                            rhs=vt[32 * i:32 * i + 4, col + C * j:col + C * j + C],
                            start=False, stop=True, skip_group_check=True,
                            tile_position=(32 * i, 32 * j))
                hv = hpool.tile([128, W4], f32, tag="hv")
                nc.scalar.activation(hv.rearrange("p (q w) -> p q w", q=4),
                                     cpv(C), RELU)
                state["hv"] = hv

            def s_rgb():
                hv = state.pop("hv")
                MR = 32 if SIM_SAFE else 3
                mmr, cpr = ps_step()
                for s in range(4):
                    for b in range(4):
                        nc.tensor.matmul(
                            out=mmr(32 * s, MR, b, C),
                            lhsT=wt[32 * b:32 * b + 32, 260 + 3 * s:260 + 3 * s + MR],
                            rhs=hv[32 * b:32 * b + 32, C * s:C * s + C],
                            start=True, stop=True, skip_group_check=True,
                            tile_position=(32 * b, 32 * s))
                nc.vector.tensor_copy(
                    otr_all[:, col:col + W4].rearrange("p (q w) -> p q w", q=4),
                    cpr(C))

            return [s_l0, s_l1, s_sigma, s_view, s_rgb]

        for base in range(0, NGROUPS, PIPE):
            window = [group_steps(g)
                      for g in range(base, min(base + PIPE, NGROUPS))]
            for stepi in range(5):
                for steps in window:
                    steps[stepi]()

        for b in range(4):
            nc.sync.dma_start(out=out_d[4 * b:4 * b + 3, :],
                              in_=otr_all[32 * b:32 * b + 3, :])
            nc.sync.dma_start(out=out_d[4 * b + 3:4 * b + 4, :],
                              in_=ots_all[32 * b:32 * b + 1, :])

    nc.compile()
    return nc


def _decode_out(results, decode, sigma_b, rgb_b):
    y = np.empty((N, 4), np.float32)
    outs = [np.asarray(r["out"]) for r in results]
    for (c, gid, pts, i, j, ca, cs, cnt) in decode:
        if cnt == 0:
            continue
        o = outs[c]
        y[pts, 0:3] = o[4 * i:4 * i + 3, ca:ca + cnt].T + rgb_b[gid]
        y[pts, 3] = o[4 * j + 3, cs:cs + cnt] + sigma_b[gid, 0]
    return y


def kernel(**inputs):
    from concourse.bass_utils import run_bass_kernel_spmd

    per_core, decode, caps, colstart, w_tot = _prep(**inputs)
    nc = _build_nc(caps, w_tot)
    in_maps = [per_core[c] for c in range(NCORES)]
    res = run_bass_kernel_spmd(nc, in_maps, list(range(NCORES)))
    return _decode_out(res.results, decode,
                       np.asarray(inputs["sigma_b"], np.float32),
                       np.asarray(inputs["rgb_b"], np.float32))


# ---------------------------------------------------------------------------
# numpy emulation of the device program (for layout validation in test.py)
def _emulate_core(arrs, caps, w_tot):
    xt = np.zeros((128, w_tot), np.float32)
    vt = np.zeros((128, w_tot), np.float32)
    for i in range(4):
        xt[32 * i:32 * i + 4] = arrs["xpts"][4 * i:4 * i + 4]
        vt[32 * i:32 * i + 4] = arrs["views"][4 * i:4 * i + 4]
    out = np.zeros((16, w_tot), np.float32)
    col = 0
    for g in range(NGROUPS):
        C = int(caps[g])
        W4 = 4 * C
        wt = arrs["wblob"][:, g * WBLOB_F:(g + 1) * WBLOB_F]
        st = np.zeros((128, SBLOB_F), np.float32)
        for i in range(4):
            st[32 * i:32 * i + 4] = arrs["sblob"][4 * i:4 * i + 4,
                                                  g * SBLOB_F:(g + 1) * SBLOB_F]

        ps0 = np.zeros((128, W4), np.float32)
        for l in range(EPG):
            i, j = l % 4, l // 4
            ps0[32 * j:32 * j + 32, C * i:C * i + C] = (
                st[32 * i:32 * i + 4, 32 * j:32 * j + 32].T
                @ xt[32 * i:32 * i + 4, col + C * j:col + C * j + C])
        h1 = np.maximum(ps0, 0)
        ps1 = np.zeros((128, W4), np.float32)
        for l in range(EPG):
            b, s = l // 4, l % 4
            ps1[32 * s:32 * s + 32, C * b:C * b + C] = (
                wt[32 * b:32 * b + 32, 32 * s:32 * s + 32].T
                @ h1[32 * b:32 * b + 32, C * s:C * s + C])
        h2 = np.empty_like(ps1)
        for q in range(4):
            h2[:, C * q:C * q + C] = np.maximum(
                ps1[:, C * q:C * q + C] + wt[:, 272 + q:273 + q], 0)
        pss = np.zeros((128, W4), np.float32)
        for l in range(EPG):
            i, j = l % 4, l // 4
            rhs = h2[32 * i:32 * i + 32, C * j:C * j + C]
            pss[32 * j:32 * j + 1, C * i:C * i + C] = (
                wt[32 * i:32 * i + 32, 256 + j:257 + j].T @ rhs)
        psv = np.zeros((128, W4), np.float32)
        for l in range(EPG):
            i, j = l % 4, l // 4
            psv[32 * j:32 * j + 32, C * i:C * i + C] = (
                wt[32 * i:32 * i + 32, 128 + 32 * j:128 + 32 * j + 32].T
                @ h2[32 * i:32 * i + 32, C * j:C * j + C]
                + st[32 * i:32 * i + 4, 128 + 32 * j:128 + 32 * j + 32].T
                @ vt[32 * i:32 * i + 4, col + C * j:col + C * j + C])
        hv = np.maximum(psv, 0)
        psr = np.zeros((128, W4), np.float32)
        for l in range(EPG):
            b, s = l // 4, l % 4
            psr[32 * s:32 * s + 3, C * b:C * b + C] = (
                wt[32 * b:32 * b + 32, 260 + 3 * s:263 + 3 * s].T
                @ hv[32 * b:32 * b + 32, C * s:C * s + C])
        for b in range(4):
            out[4 * b:4 * b + 3, col:col + W4] = psr[32 * b:32 * b + 3, :]
            out[4 * b + 3, col:col + W4] = pss[32 * b, :]
        col += W4
    return out


def kernel_emulated(**inputs):
    per_core, decode, caps, colstart, w_tot = _prep(**inputs)
    results = [{"out": _emulate_core(per_core[c], caps, w_tot)}
               for c in range(NCORES)]
    return _decode_out(results, decode,
                       np.asarray(inputs["sigma_b"], np.float32),
                       np.asarray(inputs["rgb_b"], np.float32))



# revision 14
# speedup vs baseline: 1.6737x; 1.6737x over previous
"""BatchedKiloNeRF Trainium2 kernel.

Strategy (expert-parallel, host routing):
  - 4096 tiny MLPs ("experts"), 131072 points routed by model_indices.
  - Host sorts experts by point count, packs them into groups of 16
    (8 cores x 32 groups x 16 experts). Each group of 16 experts maps onto
    the 16 independent 32x32 sub-arrays of the PE (tile_position packing):
    expert (row r, col c) reads rhs from partition band 32r and writes
    PSUM band 32c, so all 16 per-expert matmuls run concurrently.
  - HW constraint (found empirically): concurrent matmuls from different
    row groups must not write the same PSUM bank. Each step therefore uses
    a 4-bank PSUM tile [128, 2048]; row group r writes its outputs into
    bank r (column window 512*r..512*r+C). Copies back to SBUF use strided
    APs so each step needs only one ACT/DVE op.
  - Points of each expert are padded to the group capacity C (max count in
    the 128-expert window); hidden states live as [128, 4C] SBUF tiles:
    partition band = expert band, C-column segment = expert segment.
  - Layer chain alternates layouts: A (band=e%4, seg=e//4) <-> B
    (band=e//4, seg=e%4); every step uses all 16 (row, col) positions.
  - Biases: L0 and view-layer biases ride in the matmul via a constant-1
    input row (K=3 -> K=4). feat bias is folded into the view bias on the
    host. L1 bias+relu is a fused DVE tensor_scalar (add, max) per bank
    slot. sigma/rgb biases are added on host.
  - Weights are pre-permuted/transposed on host into per-group SBUF-layout
    blobs so each group needs only a handful of large DMAs.
"""

import sys

import numpy as np
import ml_dtypes

BF16 = ml_dtypes.bfloat16

for _p in ("/opt/trn_rl_repo",):
    if _p not in sys.path:
        sys.path.append(_p)

NUM_MODELS = 4096
W = 32
N = 131072
NCORES = 8
NGROUPS = 32          # expert groups per core
EPG = 16              # experts per group
WIN = NCORES * EPG    # experts per capacity window (128)

# wblob column layout (per group, [128, WBLOB_F]):
#   w1 lhsT    [  0:128)
#   viewA lhsT [128:256)   (view_w[:, :32] @ feat_w folded on host)
#   sigma lhsT [256:260)
#   rgb lhsT   [260:272)
#   b1 bias    [272:276)   (per-slot per-partition bias columns)
#   zero pad   [276:304)   (SIM_SAFE mode widens sigma/rgb lhsT windows to
#                           M=32 so the full PSUM band is written)
WBLOB_F = 304
# SIM_SAFE: write full 32-row PSUM bands for sigma/rgb so CoreSim's
# uninitialized-read checker passes; on HW the junk rows are never read and
# narrow loads (M=1 / M=3) are ~25us faster.
SIM_SAFE = False
# sblob per group [16, 256]: w0aug lhsT [0:128), viewBaug lhsT [128:256)
SBLOB_F = 256
BANK = 512            # f32 elements per PSUM bank (per partition)


def _prep(x, model_indices, pts_w0, pts_b0, pts_w1, pts_b1,
          feat_w, feat_b, sigma_w, sigma_b, view_w, view_b, rgb_w, rgb_b):
    """Host-side routing + packing. Returns per-core device arrays and
    decode info."""
    x = np.asarray(x, np.float32)
    idx = np.asarray(model_indices).astype(np.int64)
    counts = np.bincount(idx, minlength=NUM_MODELS)

    expert_order = np.argsort(-counts, kind="stable")  # descending count
    caps = np.empty(NGROUPS, np.int64)
    for k in range(NGROUPS):
        win = expert_order[WIN * k:WIN * (k + 1)]
        c = int(counts[win].max())
        caps[k] = max(4, -(-c // 4) * 4)  # round up to multiple of 4, >=4
    assert caps.max() <= BANK
    colstart = np.concatenate([[0], np.cumsum(4 * caps)])
    w_tot = int(colstart[-1])

    order_pts = np.argsort(idx, kind="stable")
    starts = np.concatenate([[0], np.cumsum(counts)])

    # fold the feat layer into the view layer on the host:
    #   view(h) = relu(Wv [feat(h); views] + bv)
    #           = relu((Wv[:, :32] @ Wf) h + WvB views + (bv + Wv[:, :32] bf))
    vb_fold = view_b + np.einsum("goh,gh->go", view_w[:, :, :W], feat_b)
    vwA_fold = np.einsum("gox,gxh->goh", view_w[:, :, :W], feat_w)
    w0aug = np.concatenate(
        [np.transpose(pts_w0, (0, 2, 1)), pts_b0[:, None, :]], axis=1
    ).astype(np.float32)                      # [E, 4, 32] lhsT rows: xyz+bias
    vwBaug = np.concatenate(
        [np.transpose(view_w[:, :, W:], (0, 2, 1)), vb_fold[:, None, :]], axis=1
    ).astype(np.float32)                      # [E, 4, 32]
    w1T = np.transpose(pts_w1, (0, 2, 1)).astype(np.float32)    # [E,32,32]
    vwAT = np.transpose(vwA_fold, (0, 2, 1)).astype(np.float32)
    sigT = np.transpose(sigma_w, (0, 2, 1)).astype(np.float32)  # [E,32,1]
    rgbT = np.transpose(rgb_w, (0, 2, 1)).astype(np.float32)    # [E,32,3]
    b1 = np.asarray(pts_b1, np.float32)

    per_core = []
    decode = []  # (core, gid, pts, row_b, col_lo, cnt)
    for c in range(NCORES):
        gq = np.stack([expert_order[WIN * k + EPG * c: WIN * k + EPG * (c + 1)]
                       for k in range(NGROUPS)])  # [32, 16]

        wblob = np.zeros((NGROUPS, 128, WBLOB_F), np.float32)
        # B-step matrices (input layout B): l = 4b+s -> [k,(b kin),(s kout)]
        arr = w1T[gq].reshape(NGROUPS, 4, 4, W, W)    # [k,b,s,kin,kout]
        wblob[:, :, 0:128] = arr.transpose(0, 1, 3, 2, 4).reshape(NGROUPS, 128, 128)
        arr = vwAT[gq].reshape(NGROUPS, 4, 4, W, W)  # [k,j,i,kin,kout] (A)
        wblob[:, :, 128:256] = arr.transpose(0, 2, 3, 1, 4).reshape(NGROUPS, 128, 128)
        # A-step matrices (input layout A): l = 4j+i -> [k,(i kin),(j kout)]
        arr = sigT[gq].reshape(NGROUPS, 4, 4, W, 1)
        wblob[:, :, 256:260] = arr.transpose(0, 2, 3, 1, 4).reshape(NGROUPS, 128, 4)
        arr = rgbT[gq].reshape(NGROUPS, 4, 4, W, 3)   # [k,b,s,kin,kout] (B)
        wblob[:, :, 260:272] = arr.transpose(0, 1, 3, 2, 4).reshape(NGROUPS, 128, 12)
        # b1: L1 out layout A: partition 32s+h (s=e%4), slot q=e//4 -> col q
        arr = b1[gq].reshape(NGROUPS, 4, 4, W)        # [k,q,s,h]
        wblob[:, :, 272:276] = arr.transpose(0, 2, 3, 1).reshape(NGROUPS, 128, 4)

        sblob = np.zeros((NGROUPS, 16, SBLOB_F), np.float32)
        arr = w0aug[gq].reshape(NGROUPS, 4, 4, 4, W)  # [k,j,i,kin,kout] (A)
        sblob[:, :, 0:128] = arr.transpose(0, 2, 3, 1, 4).reshape(NGROUPS, 16, 128)
        arr = vwBaug[gq].reshape(NGROUPS, 4, 4, 4, W)  # [k,j,i,kin,kout] (A)
        sblob[:, :, 128:256] = arr.transpose(0, 2, 3, 1, 4).reshape(NGROUPS, 16, 128)

        xpts = np.zeros((16, w_tot), np.float32)
        views = np.zeros((16, w_tot), np.float32)
        xpts[3::4, :] = 1.0   # constant-1 rows for bias-in-matmul
        views[3::4, :] = 1.0
        for k in range(NGROUPS):
            C = int(caps[k])
            for l in range(EPG):
                gid = int(gq[k, l])
                cnt = int(counts[gid])
                pts = order_pts[starts[gid]:starts[gid] + cnt]
                i, j = l % 4, l // 4      # layout A (band, seg)
                ca = int(colstart[k]) + C * j   # A-seg columns
                cs = int(colstart[k]) + C * i   # B-seg columns
                if cnt:
                    xv = x[pts]
                    xpts[4 * i:4 * i + 3, ca:ca + cnt] = xv[:, :3].T
                    views[4 * i:4 * i + 3, ca:ca + cnt] = xv[:, 3:6].T
                # rgb lands at rows 4i+0..2, cols ca; sigma at row 4j+3, cols cs
                decode.append((c, gid, pts, i, j, ca, cs, cnt))
        per_core.append(dict(
            xpts=xpts.astype(BF16), views=views.astype(BF16),
            wblob=wblob.transpose(1, 0, 2).reshape(128, NGROUPS * WBLOB_F)
                       .astype(BF16),
            sblob=sblob.transpose(1, 0, 2).reshape(16, NGROUPS * SBLOB_F)
                       .astype(BF16)))

    b1_zero = not np.any(b1)
    return per_core, decode, caps, colstart, w_tot, b1_zero


def _build_nc(caps, w_tot, b1_zero):
    import concourse.mybir as mybir
    import concourse.tile as tile
    from concourse import bacc
    from contextlib import ExitStack

    f32 = mybir.dt.float32
    bf16 = mybir.dt.bfloat16
    RELU = mybir.ActivationFunctionType.Relu
    ADD = mybir.AluOpType.add
    MAX = mybir.AluOpType.max

    nc = bacc.Bacc("TRN2", target_bir_lowering=False)
    xpts_d = nc.declare_dram_parameter("xpts", [16, w_tot], bf16, isOutput=False)
    views_d = nc.declare_dram_parameter("views", [16, w_tot], bf16,
                                        isOutput=False)
    wblob_d = nc.declare_dram_parameter("wblob", [128, NGROUPS * WBLOB_F], bf16,
                                        isOutput=False)
    sblob_d = nc.declare_dram_parameter("sblob", [16, NGROUPS * SBLOB_F], bf16,
                                        isOutput=False)
    out_d = nc.declare_dram_parameter("out", [16, w_tot], f32, isOutput=True)

    with tile.TileContext(nc) as tc, ExitStack() as ctx:
        const = ctx.enter_context(tc.tile_pool(name="const", bufs=1))
        hpool = ctx.enter_context(tc.tile_pool(name="h", bufs=8))
        pspool = ctx.enter_context(tc.tile_pool(name="ps", bufs=1, space="PSUM"))
        # One persistent 8-bank PSUM tensor, hand-slotted: a step claims
        # (bank-set, column-offset) slot; bank q within the set = row group q
        # (different row groups must not share a bank; same row group may).
        psall = pspool.tile([128, 8 * BANK], f32, tag="psall")
        step_ctr = [0]

        def ps_step():
            sidx = step_ctr[0]
            step_ctr[0] += 1
            bs = sidx % 2
            co = ((sidx // 2) % 8) * 64

            def mm_out(part_lo, m, q, C):
                base = (4 * bs + q) * BANK + co
                return psall[part_lo:part_lo + m, base:base + C]

            def copy_src(C):
                return psall.rearrange("p (b w) -> p b w", b=8)[
                    :, 4 * bs:4 * bs + 4, co:co + C]

            return mm_out, copy_src

        xt = const.tile([128, w_tot], bf16)
        vt = const.tile([128, w_tot], bf16)
        for i in range(4):
            nc.sync.dma_start(out=xt[32 * i:32 * i + 4, :],
                              in_=xpts_d[4 * i:4 * i + 4, :])
            nc.sync.dma_start(out=vt[32 * i:32 * i + 4, :],
                              in_=views_d[4 * i:4 * i + 4, :])
        wt_all = const.tile([128, NGROUPS * WBLOB_F], bf16)
        wtot = NGROUPS * WBLOB_F
        nchunk = 8
        csz = -(-wtot // nchunk)
        for u in range(nchunk):
            lo, hi = u * csz, min((u + 1) * csz, wtot)
            nc.sync.dma_start(out=wt_all[:, lo:hi], in_=wblob_d[:, lo:hi])
        st_all = const.tile([128, NGROUPS * SBLOB_F], bf16)
        for i in range(4):
            nc.sync.dma_start(out=st_all[32 * i:32 * i + 4, :],
                              in_=sblob_d[4 * i:4 * i + 4, :])
        otr_all = const.tile([128, w_tot], f32)
        ots_all = const.tile([128, w_tot], f32)

        # Software-pipeline: emit steps step-major over windows of PIPE
        # groups so the PE always has another group's matmuls to run while
        # a step's PSUM->SBUF copy completes.
        PIPE = 4
        colstarts = np.concatenate([[0], np.cumsum(4 * np.asarray(caps))])

        def group_steps(g):
            C = int(caps[g])
            W4 = 4 * C
            col = int(colstarts[g])
            wt = wt_all[:, g * WBLOB_F:(g + 1) * WBLOB_F]
            st = st_all[:, g * SBLOB_F:(g + 1) * SBLOB_F]
            state = {}

            def s_l0():
                mm0, cp0 = ps_step()
                for j in range(4):
                    for i in range(4):
                        nc.tensor.matmul(
                            out=mm0(32 * j, 32, i, C),
                            lhsT=st[32 * i:32 * i + 4, 32 * j:32 * j + 32],
                            rhs=xt[32 * i:32 * i + 4, col + C * j:col + C * j + C],
                            start=True, stop=True, skip_group_check=True,
                            tile_position=(32 * i, 32 * j))
                h1 = hpool.tile([128, W4], bf16, tag="h1")
                nc.scalar.activation(h1.rearrange("p (q w) -> p q w", q=4),
                                     cp0(C), RELU)
                state["h1"] = h1

            def s_l1():
                h1 = state.pop("h1")
                mm1, cp1 = ps_step()
                for s in range(4):
                    for b in range(4):
                        nc.tensor.matmul(
                            out=mm1(32 * s, 32, b, C),
                            lhsT=wt[32 * b:32 * b + 32, 32 * s:32 * s + 32],
                            rhs=h1[32 * b:32 * b + 32, C * s:C * s + C],
                            start=True, stop=True, skip_group_check=True,
                            tile_position=(32 * b, 32 * s))
                h2 = hpool.tile([128, W4], bf16, tag="h2")
                if b1_zero:
                    nc.vector.tensor_scalar_max(
                        h2.rearrange("p (q w) -> p q w", q=4), cp1(C), 0.0)
                else:
                    for q in range(4):
                        nc.vector.tensor_scalar(
                            out=h2[:, C * q:C * q + C],
                            in0=mm1(0, 128, q, C),
                            scalar1=wt[:, 272 + q:273 + q], scalar2=0.0,
                            op0=ADD, op1=MAX)
                state["h2"] = h2

            def s_sigma():
                h2 = state["h2"]
                MS = 32 if SIM_SAFE else 1
                mms_, cps = ps_step()
                for j in range(4):
                    for i in range(4):
                        nc.tensor.matmul(
                            out=mms_(32 * j, MS, i, C),
                            lhsT=wt[32 * i:32 * i + 32, 256 + j:256 + j + MS],
                            rhs=h2[32 * i:32 * i + 32, C * j:C * j + C],
                            start=True, stop=True, skip_group_check=True,
                            tile_position=(32 * i, 32 * j))
                nc.vector.tensor_copy(
                    ots_all[:, col:col + W4].rearrange("p (q w) -> p q w", q=4),
                    cps(C))

            def s_view():
                h2 = state.pop("h2")
                mmv, cpv = ps_step()
                for j in range(4):
                    for i in range(4):
                        nc.tensor.matmul(
                            out=mmv(32 * j, 32, i, C),
                            lhsT=wt[32 * i:32 * i + 32, 128 + 32 * j:128 + 32 * j + 32],
                            rhs=h2[32 * i:32 * i + 32, C * j:C * j + C],
                            start=True, stop=False, skip_group_check=True,
                            tile_position=(32 * i, 32 * j))
                    for i in range(4):
                        nc.tensor.matmul(
                            out=mmv(32 * j, 32, i, C),
                            lhsT=st[32 * i:32 * i + 4, 128 + 32 * j:128 + 32 * j + 32],
                            rhs=vt[32 * i:32 * i + 4, col + C * j:col + C * j + C],
                            start=False, stop=True, skip_group_check=True,
                            tile_position=(32 * i, 32 * j))
                hv = hpool.tile([128, W4], bf16, tag="hv")
                nc.scalar.activation(hv.rearrange("p (q w) -> p q w", q=4),
                                     cpv(C), RELU)
                state["hv"] = hv

            def s_rgb():
                hv = state.pop("hv")
                MR = 32 if SIM_SAFE else 3
                mmr, cpr = ps_step()
                for s in range(4):
                    for b in range(4):
                        nc.tensor.matmul(
                            out=mmr(32 * s, MR, b, C),
                            lhsT=wt[32 * b:32 * b + 32, 260 + 3 * s:260 + 3 * s + MR],
                            rhs=hv[32 * b:32 * b + 32, C * s:C * s + C],
                            start=True, stop=True, skip_group_check=True,
                            tile_position=(32 * b, 32 * s))
                nc.vector.tensor_copy(
                    otr_all[:, col:col + W4].rearrange("p (q w) -> p q w", q=4),
                    cpr(C))

            return [s_l0, s_l1, s_sigma, s_view, s_rgb]

        for base in range(0, NGROUPS, PIPE):
            window = [group_steps(g)
                      for g in range(base, min(base + PIPE, NGROUPS))]
            for stepi in range(5):
                for steps in window:
                    steps[stepi]()

        for b in range(4):
            nc.sync.dma_start(out=out_d[4 * b:4 * b + 3, :],
                              in_=otr_all[32 * b:32 * b + 3, :])
            nc.sync.dma_start(out=out_d[4 * b + 3:4 * b + 4, :],
                              in_=ots_all[32 * b:32 * b + 1, :])

    nc.compile()
    return nc


def _decode_out(results, decode, sigma_b, rgb_b):
    y = np.empty((N, 4), np.float32)
    outs = [np.asarray(r["out"]) for r in results]
    for (c, gid, pts, i, j, ca, cs, cnt) in decode:
        if cnt == 0:
            continue
        o = outs[c]
        y[pts, 0:3] = o[4 * i:4 * i + 3, ca:ca + cnt].T + rgb_b[gid]
        y[pts, 3] = o[4 * j + 3, cs:cs + cnt] + sigma_b[gid, 0]
    return y


def kernel(**inputs):
    from concourse.bass_utils import run_bass_kernel_spmd

    per_core, decode, caps, colstart, w_tot, b1_zero = _prep(**inputs)
    nc = _build_nc(caps, w_tot, b1_zero)
    in_maps = [per_core[c] for c in range(NCORES)]
    res = run_bass_kernel_spmd(nc, in_maps, list(range(NCORES)))
    return _decode_out(res.results, decode,
                       np.asarray(inputs["sigma_b"], np.float32),
                       np.asarray(inputs["rgb_b"], np.float32))


# ---------------------------------------------------------------------------
# numpy emulation of the device program (for layout validation in test.py)
def _emulate_core(arrs, caps, w_tot):
    arrs = {k: np.asarray(v, np.float32) for k, v in arrs.items()}
    xt = np.zeros((128, w_tot), np.float32)
    vt = np.zeros((128, w_tot), np.float32)
    for i in range(4):
        xt[32 * i:32 * i + 4] = arrs["xpts"][4 * i:4 * i + 4]
        vt[32 * i:32 * i + 4] = arrs["views"][4 * i:4 * i + 4]
    out = np.zeros((16, w_tot), np.float32)
    col = 0
    for g in range(NGROUPS):
        C = int(caps[g])
        W4 = 4 * C
        wt = arrs["wblob"][:, g * WBLOB_F:(g + 1) * WBLOB_F]
        st = np.zeros((128, SBLOB_F), np.float32)
        for i in range(4):
            st[32 * i:32 * i + 4] = arrs["sblob"][4 * i:4 * i + 4,
                                                  g * SBLOB_F:(g + 1) * SBLOB_F]

        ps0 = np.zeros((128, W4), np.float32)
        for l in range(EPG):
            i, j = l % 4, l // 4
            ps0[32 * j:32 * j + 32, C * i:C * i + C] = (
                st[32 * i:32 * i + 4, 32 * j:32 * j + 32].T
                @ xt[32 * i:32 * i + 4, col + C * j:col + C * j + C])
        h1 = np.maximum(ps0, 0)
        ps1 = np.zeros((128, W4), np.float32)
        for l in range(EPG):
            b, s = l // 4, l % 4
            ps1[32 * s:32 * s + 32, C * b:C * b + C] = (
                wt[32 * b:32 * b + 32, 32 * s:32 * s + 32].T
                @ h1[32 * b:32 * b + 32, C * s:C * s + C])
        h2 = np.empty_like(ps1)
        for q in range(4):
            h2[:, C * q:C * q + C] = np.maximum(
                ps1[:, C * q:C * q + C] + wt[:, 272 + q:273 + q], 0)
        pss = np.zeros((128, W4), np.float32)
        for l in range(EPG):
            i, j = l % 4, l // 4
            rhs = h2[32 * i:32 * i + 32, C * j:C * j + C]
            pss[32 * j:32 * j + 1, C * i:C * i + C] = (
                wt[32 * i:32 * i + 32, 256 + j:257 + j].T @ rhs)
        psv = np.zeros((128, W4), np.float32)
        for l in range(EPG):
            i, j = l % 4, l // 4
            psv[32 * j:32 * j + 32, C * i:C * i + C] = (
                wt[32 * i:32 * i + 32, 128 + 32 * j:128 + 32 * j + 32].T
                @ h2[32 * i:32 * i + 32, C * j:C * j + C]
                + st[32 * i:32 * i + 4, 128 + 32 * j:128 + 32 * j + 32].T
                @ vt[32 * i:32 * i + 4, col + C * j:col + C * j + C])
        hv = np.maximum(psv, 0)
        psr = np.zeros((128, W4), np.float32)
        for l in range(EPG):
            b, s = l // 4, l % 4
            psr[32 * s:32 * s + 3, C * b:C * b + C] = (
                wt[32 * b:32 * b + 32, 260 + 3 * s:263 + 3 * s].T
                @ hv[32 * b:32 * b + 32, C * s:C * s + C])
        for b in range(4):
            out[4 * b:4 * b + 3, col:col + W4] = psr[32 * b:32 * b + 3, :]
            out[4 * b + 3, col:col + W4] = pss[32 * b, :]
        col += W4
    return out


def kernel_emulated(**inputs):
    per_core, decode, caps, colstart, w_tot, b1_zero = _prep(**inputs)
    results = [{"out": _emulate_core(per_core[c], caps, w_tot)}
               for c in range(NCORES)]
    return _decode_out(results, decode,
                       np.asarray(inputs["sigma_b"], np.float32),
                       np.asarray(inputs["rgb_b"], np.float32))

